# revision 3
# baseline (speedup 1.0000x reference)
"""Additive (Bahdanau) attention on 8 TRN2 NeuronCores.

Problem shapes (hardcoded): B=4, n=512, m=1024, dq=dk=dv=256, h=128.
Sharding: data-parallel over (batch, n-half) -> 8 independent shards, one per
core, no collectives. Each core computes 256 query rows against its batch's
1024 keys/values.

Per-core algorithm (all layouts partition-major on h=128 where possible):
  tqT[h, i] = Wq.T @ qT + (bq+bk)      (PE, f32)
  tkT[h, j] = Wk.T @ kT                (PE, f32)
  for each block of IB query rows:
    S[h, il, j] = tkT[h, j] + tqT[h, i]    (DVE tensor_scalar, f32)
    T = tanh(S)                            (ACT, one instr per block, bf16 out)
    score[i, j] += wv . T[:, il, j]  via accumulating matmuls whose lhsT is a
        sliding window of a (h, 256) matrix holding wv in column 128, so row i
        of PSUM receives wv.T @ T_i       (PE, bf16)
  per 128-row group: mask-add, exp (+row-sum via accum_out), transpose the
  bf16 weights, weight @ values, scale by 1/rowsum, DMA out.
"""

import numpy as np

import concourse.bass as bass
import concourse.mybir as mybir
import concourse.tile as tile
from concourse import bacc
from concourse.bass_utils import run_bass_kernel_spmd
from concourse.masks import make_identity

F32 = mybir.dt.float32
BF16 = mybir.dt.bfloat16

B, N, M = 4, 512, 1024
DQ, DK, DV, H = 256, 256, 256, 128
N_CORES = 8
N_LOC = B * N // N_CORES  # 256 query rows per core
IB = 8                    # query rows per tanh block
NEG = -40.0               # additive mask value (exp(-40+score) == 0 vs valid weights)


def build_nc():
    nc = bacc.Bacc("TRN2", target_bir_lowering=False)

    qT_d = nc.declare_dram_parameter("qT", [DQ, N_LOC], F32, isOutput=False)
    kT_d = nc.declare_dram_parameter("kT", [DK, M], F32, isOutput=False)
    v_d = nc.declare_dram_parameter("v", [M, DV], F32, isOutput=False)
    badd_d = nc.declare_dram_parameter("badd", [N_LOC, M], F32, isOutput=False)
    wq_d = nc.declare_dram_parameter("Wq", [DQ, H], F32, isOutput=False)
    wk_d = nc.declare_dram_parameter("Wk", [DK, H], F32, isOutput=False)
    wv_d = nc.declare_dram_parameter("wv", [H, 1], F32, isOutput=False)
    bqk_d = nc.declare_dram_parameter("bqk", [H, 1], F32, isOutput=False)
    out_d = nc.declare_dram_parameter("out", [N_LOC, DV], F32, isOutput=True)

    tanh = mybir.ActivationFunctionType.Tanh
    expf = mybir.ActivationFunctionType.Exp

    with tile.TileContext(nc) as tc:
        with tc.tile_pool(name="const", bufs=1) as cpool:
            # ---- constants / inputs resident for the whole kernel ----
            wv_sb = cpool.tile([H, 1], F32)
            nc.sync.dma_start(wv_sb[:, :], wv_d[:, :])
            bqk_sb = cpool.tile([H, 1], F32)
            nc.sync.dma_start(bqk_sb[:, :], bqk_d[:, :])

            # sliding-window lhsT: wv lives in column 128; slice [128-i, 256-i)
            # puts wv in window-column i, zeros elsewhere.
            msl = cpool.tile([H, 2 * H], BF16)
            nc.gpsimd.memset(msl[:, :], 0.0)
            nc.vector.tensor_copy(msl[:, H : H + 1], wv_sb[:, :])

            ident = cpool.tile([H, H], BF16)
            make_identity(nc, ident[:, :])

            kt_sb = cpool.tile([128, 2, M], F32)
            nc.sync.dma_start(kt_sb[:, :, :], kT_d.rearrange("(t p) j -> p t j", p=128))
            qt_sb = cpool.tile([128, 2, N_LOC], F32)
            nc.sync.dma_start(qt_sb[:, :, :], qT_d.rearrange("(t p) i -> p t i", p=128))
            wq_sb = cpool.tile([128, 2, H], F32)
            nc.sync.dma_start(wq_sb[:, :, :], wq_d.rearrange("(t p) h -> p t h", p=128))
            wk_sb = cpool.tile([128, 2, H], F32)
            nc.sync.dma_start(wk_sb[:, :, :], wk_d.rearrange("(t p) h -> p t h", p=128))

            v_f32 = cpool.tile([128, M // 128, DV], F32)
            nc.sync.dma_start(v_f32[:, :, :], v_d.rearrange("(t p) v -> p t v", p=128))
            v_bf = cpool.tile([128, M // 128, DV], BF16)
            nc.vector.tensor_copy(v_bf[:, :, :], v_f32[:, :, :])

            badd_sb = cpool.tile([128, N_LOC // 128, M], F32)
            nc.sync.dma_start(
                badd_sb[:, :, :], badd_d.rearrange("(t p) j -> p t j", p=128)
            )

            tqT_sb = cpool.tile([H, N_LOC], F32)
            tkT_sb = cpool.tile([H, M], F32)

            # ---- transformed queries/keys (f32 PE matmuls) ----
            with tc.tile_pool(name="setup_psum", bufs=2, space=bass.MemorySpace.PSUM) as spp:
                tq_ps = spp.tile([H, N_LOC], F32)
                for t in range(2):
                    nc.tensor.matmul(
                        tq_ps[:, :], wq_sb[:, t, :], qt_sb[:, t, :],
                        start=(t == 0), stop=(t == 1),
                    )
                # fold bq+bk into the query transform
                nc.vector.tensor_scalar_add(tqT_sb[:, :], tq_ps[:, :], bqk_sb[:, 0:1])

                for jh in range(2):
                    tk_ps = spp.tile([H, 512], F32)
                    for t in range(2):
                        nc.tensor.matmul(
                            tk_ps[:, :], wk_sb[:, t, :],
                            kt_sb[:, t, jh * 512 : (jh + 1) * 512],
                            start=(t == 0), stop=(t == 1),
                        )
                    nc.vector.tensor_copy(tkT_sb[:, jh * 512 : (jh + 1) * 512], tk_ps[:, :])

            # ---- main pipeline ----
            with (
                tc.tile_pool(name="s_pool", bufs=2) as s_pool,
                tc.tile_pool(name="t_pool", bufs=2) as t_pool,
                tc.tile_pool(name="sm_pool", bufs=2) as sm_pool,
                tc.tile_pool(name="w_pool", bufs=2) as w_pool,
                tc.tile_pool(name="wt_pool", bufs=2) as wt_pool,
                tc.tile_pool(name="o_pool", bufs=2) as o_pool,
                tc.tile_pool(name="stat", bufs=4) as stat,
                tc.tile_pool(name="score_ps", bufs=4, space=bass.MemorySpace.PSUM) as score_pp,
                tc.tile_pool(name="wt_ps", bufs=2, space=bass.MemorySpace.PSUM) as wt_pp,
                tc.tile_pool(name="out_ps", bufs=2, space=bass.MemorySpace.PSUM) as out_pp,
            ):
                for g in range(N_LOC // 128):
                    sc = [
                        score_pp.tile([128, 512], F32, tag="sc", name=f"sc{g}_{jh}")
                        for jh in range(2)
                    ]
                    for blk in range(128 // IB):
                        S = s_pool.tile([128, IB, M], F32)
                        for il in range(IB):
                            i = g * 128 + blk * IB + il
                            nc.vector.tensor_scalar_add(
                                S[:, il, :], tkT_sb[:, :], tqT_sb[:, i : i + 1]
                            )
                        T = t_pool.tile([128, IB, M], BF16)
                        nc.scalar.activation(T[:, :, :], S[:, :, :], tanh)
                        for il in range(IB):
                            ig = blk * IB + il
                            for jh in range(2):
                                nc.tensor.matmul(
                                    sc[jh][:, :],
                                    msl[:, H - ig : 2 * H - ig],
                                    T[:, il, jh * 512 : (jh + 1) * 512],
                                    start=(ig == 0), stop=(ig == 127),
                                )

                    # ---- softmax + output for this 128-row group ----
                    scm = sm_pool.tile([128, M], F32)
                    for jh in range(2):
                        nc.vector.tensor_add(
                            scm[:, jh * 512 : (jh + 1) * 512],
                            sc[jh][:, :],
                            badd_sb[:, g, jh * 512 : (jh + 1) * 512],
                        )
                    wexp = w_pool.tile([128, M], BF16)
                    rowsum = stat.tile([128, 1], F32)
                    nc.scalar.activation(
                        wexp[:, :], scm[:, :], expf, accum_out=rowsum[:, 0:1]
                    )
                    recip = stat.tile([128, 1], F32)
                    nc.vector.reciprocal(recip[:, 0:1], rowsum[:, 0:1])

                    wt_sb = wt_pool.tile([128, M // 128, 128], BF16)
                    for jt in range(M // 128):
                        wt_ps = wt_pp.tile([128, 128], BF16)
                        nc.tensor.transpose(
                            wt_ps[:, :], wexp[:, jt * 128 : (jt + 1) * 128], ident[:, :]
                        )
                        nc.vector.tensor_copy(wt_sb[:, jt, :], wt_ps[:, :])

                    out_ps = out_pp.tile([128, DV], F32)
                    for jt in range(M // 128):
                        nc.tensor.matmul(
                            out_ps[:, :], wt_sb[:, jt, :], v_bf[:, jt, :],
                            start=(jt == 0), stop=(jt == M // 128 - 1),
                        )
                    out_sb = o_pool.tile([128, DV], F32)
                    nc.vector.tensor_scalar_mul(out_sb[:, :], out_ps[:, :], recip[:, 0:1])
                    nc.sync.dma_start(out_d[g * 128 : (g + 1) * 128, :], out_sb[:, :])

    nc.compile()
    return nc


_NC_CACHE = []


def _get_nc():
    if not _NC_CACHE:
        _NC_CACHE.append(build_nc())
    return _NC_CACHE[0]


def make_in_maps(queries, keys, values, mask, Wq, bq, Wk, bk, wv, bv):
    f32 = np.float32
    badd_full = (mask.astype(f32) - 1.0) * -NEG  # 0 where valid, NEG where masked
    wv_col = np.ascontiguousarray(wv.reshape(H, 1).astype(f32))
    bqk = np.ascontiguousarray((bq + bk).reshape(H, 1).astype(f32))
    wq = np.ascontiguousarray(Wq.astype(f32))
    wk = np.ascontiguousarray(Wk.astype(f32))
    in_maps = []
    for c in range(N_CORES):
        b, half = divmod(c, 2)
        rows = slice(half * N_LOC, (half + 1) * N_LOC)
        in_maps.append(
            {
                "qT": np.ascontiguousarray(queries[b, rows].T.astype(f32)),
                "kT": np.ascontiguousarray(keys[b].T.astype(f32)),
                "v": np.ascontiguousarray(values[b].astype(f32)),
                "badd": np.ascontiguousarray(badd_full[b, rows]),
                "Wq": wq,
                "Wk": wk,
                "wv": wv_col,
                "bqk": bqk,
                "out": None,  # placeholder, removed below
            }
        )
        del in_maps[-1]["out"]
    return in_maps


def gather_out(results):
    out = np.zeros((B, N, DV), np.float32)
    for c in range(N_CORES):
        b, half = divmod(c, 2)
        out[b, half * N_LOC : (half + 1) * N_LOC] = results[c]["out"]
    return out


def kernel(**inputs):
    nc = _get_nc()
    in_maps = make_in_maps(**inputs)
    res = run_bass_kernel_spmd(nc, in_maps, core_ids=list(range(N_CORES)))
    return gather_out(res.results)
